# revision 31
# baseline (speedup 1.0000x reference)
"""Trainium2 Bass kernel for pre-LN multi-head attention (B=4, T=2048, D=1024, H=16).

Sharding (8 NeuronCores): core c handles batch c//2 and heads
[8*(c%2), 8*(c%2)+8).  Data-parallel over B (x2 TP over the 16 heads).
Each core computes a partial out-projection over its 512 inner dims; the
two partials per batch are summed on the host during the gather.

Device-side math per core (T=2048, D=1024, 8 local heads):
  1. LayerNorm (gamma folded into w_qkv on host, beta folded into a
     per-column qkv bias) in row-major layout, then PE-transpose to xnT
     [D on partitions, T free].
  2. QKV projection: Qt/Kt produced transposed ([head*64 dims on
     partitions, T free]); V in natural [T, inner] layout with an extra
     ones-column per head (softmax denominator trick).
  3. Attention per head in transposed form: St = K @ Q^T tiles
     [128 keys, 512 queries]; exp on ScalarE (x0.125 scale folded in);
     Ot[65, 512] += V~^T @ Pt accumulated over key blocks in PSUM; row 64
     of Ot is the softmax denominator.  Divide rows 0..63 by row 64.
  4. Out-projection straight from the transposed Ot layout.
Matmul operands are stored in bf16 (PE full rate); all accumulation,
LayerNorm and softmax statistics stay in fp32.
"""

import os
import sys

import numpy as np

for _p in ("/opt/trn_rl_repo", "/opt/pypackages"):
    if _p not in sys.path and os.path.isdir(_p):
        sys.path.append(_p)

from contextlib import ExitStack

from concourse import bacc, bass, bass_utils, masks, mybir, tile
from concourse._compat import with_exitstack
from concourse.mybir import ActivationFunctionType as AF
from concourse.mybir import AluOpType as ALU

F32 = mybir.dt.float32
BF16 = mybir.dt.bfloat16
AX = mybir.AxisListType

D = 1024          # model dim
HL = 8            # heads per core
DH = 64           # head dim
IL = HL * DH      # local inner dim = 512
EPS = 1e-5
SCALE = DH ** -0.5


def build_graph(T=2048, n_devices=8):
    """Build and compile the per-core Bass graph (same graph on all cores)."""
    CH = 512               # phase-1 token chunk
    NCH = T // CH
    NTB = T // 128         # 128-token blocks
    QB = 512               # query block
    NQB = T // QB
    NKB = T // 128         # key blocks

    nc = bacc.Bacc(
        "TRN2",
        target_bir_lowering=False,
        debug=False,
        enable_asserts=False,
        num_devices=n_devices,
    )

    x_d = nc.dram_tensor("x", [T, D], F32, kind="ExternalInput").ap()
    wq_d = nc.dram_tensor("wq", [D, IL], F32, kind="ExternalInput").ap()
    wk_d = nc.dram_tensor("wk", [D, IL], F32, kind="ExternalInput").ap()
    wv_d = nc.dram_tensor("wv", [D, IL], F32, kind="ExternalInput").ap()
    wo_d = nc.dram_tensor("wo", [IL, D], F32, kind="ExternalInput").ap()
    qb_d = nc.dram_tensor("qb", [4, 128], F32, kind="ExternalInput").ap()
    kb_d = nc.dram_tensor("kb", [4, 128], F32, kind="ExternalInput").ap()
    vb_d = nc.dram_tensor("vb", [1, IL], F32, kind="ExternalInput").ap()
    out_d = nc.dram_tensor("out", [T, D], F32, kind="ExternalOutput").ap()

    with tile.TileContext(nc) as tc:
        _build_tile(tc, x_d, wq_d, wk_d, wv_d, wo_d, qb_d, kb_d, vb_d, out_d,
                    T=T, CH=CH, NCH=NCH, NTB=NTB, QB=QB, NQB=NQB, NKB=NKB)

    nc.compile()
    return nc


@with_exitstack
def _build_tile(ctx: ExitStack, tc, x_d, wq_d, wk_d, wv_d, wo_d, qb_d, kb_d,
                vb_d, out_d, *, T, CH, NCH, NTB, QB, NQB, NKB):
    nc = tc.nc
    SUBS = CH // 128

    # ---- persistent tiles (live across all phases) ----
    pers = ctx.enter_context(tc.tile_pool(name="pers", bufs=1))
    qt_sb = pers.tile([128, 4, T], BF16)     # heads 2m,2m+1 on partitions
    kt_sb = pers.tile([128, 4, T], BF16)
    v_sb = pers.tile([128, NTB, HL * 65], BF16)  # per key block: 8x(64 V + 1 ones)
    ot_sb = pers.tile([128, 4, T], BF16)     # normalized attention out (transposed)
    ident = pers.tile([128, 128], BF16)
    qb_sb = pers.tile([128, 4], F32)
    kb_sb = pers.tile([128, 4], F32)
    vb_row = pers.tile([1, IL], F32)
    vb_bc = pers.tile([128, IL], F32)

    masks.make_identity(nc, ident[:])
    nc.sync.dma_start(qb_sb[:], qb_d.rearrange("a p -> p a"))
    nc.sync.dma_start(kb_sb[:], kb_d.rearrange("a p -> p a"))
    nc.sync.dma_start(vb_row[:], vb_d[:])
    nc.gpsimd.partition_broadcast(vb_bc[:], vb_row[:], channels=128)
    # ones columns for the softmax denominators
    v_ones = v_sb.rearrange("p b (h c) -> p b h c", c=65)[:, :, :, 64:65]
    nc.vector.memset(v_ones, 1.0)

    # ================= phase 1: LN -> xnT -> QKV =================
    p1_cm = tc.tile_pool(name="p1", bufs=1)
    p1 = p1_cm.__enter__()
    p1ps_cm = tc.tile_pool(name="p1ps", bufs=1, space="PSUM")
    p1ps = p1ps_cm.__enter__()

    # x tiles for the first chunk are on the DMA critical path: issue them
    # before the big weight loads so LN/transpose work starts immediately.
    xs_pre = []
    for sub in range(SUBS):
        xs = p1.tile([128, D], F32, tag="xs", bufs=SUBS + 2, name=f"xs_pre{sub}")
        nc.sync.dma_start(xs[:], x_d[sub * 128:(sub + 1) * 128, :])
        xs_pre.append(xs)

    wq_sb = p1.tile([128, 8, IL], BF16)
    wk_sb = p1.tile([128, 8, IL], BF16)
    wv_sb = p1.tile([128, 8, IL], BF16)
    with tc.tile_pool(name="pw", bufs=1) as pw:
        # wv first (the V projection consumes it earliest); per-slice converts
        # so the scheduler can interleave LN work on ScalarE between them
        # wv: per-slice DMA + convert so the first V matmul only waits on
        # one 256KB transfer instead of the whole 2MB weight
        wv_r = wv_d.rearrange("(a p) m -> p a m", p=128)
        for ds in range(8):
            wvst = pw.tile([128, 1, IL], F32, tag="wvst", bufs=3)
            nc.sync.dma_start(wvst[:], wv_r[:, ds:ds + 1, :])
            nc.scalar.copy(wv_sb[:, ds:ds + 1, :], wvst[:])
        for w_d, w_sb in ((wq_d, wq_sb), (wk_d, wk_sb)):
            wst = pw.tile([128, 8, IL], F32, tag="wst", bufs=2)
            nc.sync.dma_start(wst[:], w_d.rearrange("(a p) m -> p a m", p=128))
            nc.scalar.copy(w_sb[:], wst[:])

    for c in range(NCH):
        xnt = p1.tile([128, 8, CH], BF16, tag="xnt", bufs=3)
        for sub in range(SUBS):
            r0 = c * CH + sub * 128
            if c == 0:
                xs = xs_pre[sub]
            else:
                xs = p1.tile([128, D], F32, tag="xs", bufs=SUBS + 2)
                nc.sync.dma_start(xs[:], x_d[r0:r0 + 128, :])
            # per-tile LN stats: mu = sx/D; var = sx2/D - mu^2 + eps
            mu = p1.tile([128, 1], F32, tag="mu", bufs=3)
            nc.vector.reduce_sum(mu[:], xs[:], axis=AX.X)
            sq = p1.tile([128, D], F32, tag="sq", bufs=2)
            var = p1.tile([128, 1], F32, tag="var", bufs=3)
            nc.scalar.activation(sq[:], xs[:], AF.Square, accum_out=var[:])
            nc.vector.tensor_scalar_mul(mu[:], mu[:], 1.0 / D)
            mu2 = p1.tile([128, 1], F32, tag="mu2", bufs=3)
            nc.vector.scalar_tensor_tensor(
                mu2[:], mu[:], 1.0, mu[:], op0=ALU.mult, op1=ALU.mult)
            nc.vector.tensor_scalar(
                var[:], var[:], 1.0 / D, EPS, op0=ALU.mult, op1=ALU.add)
            nc.vector.tensor_sub(var[:], var[:], mu2[:])
            sd = p1.tile([128, 1], F32, tag="sd", bufs=3)
            nc.scalar.activation(sd[:], var[:], AF.Sqrt)
            rstd = p1.tile([128, 1], F32, tag="rstd", bufs=3)
            nc.vector.reciprocal(rstd[:], sd[:])
            # --- normalize (bf16) + transpose into xnt ---
            zb = p1.tile([128, D], BF16, tag="zb", bufs=2)
            nc.vector.tensor_scalar(
                zb[:], xs[:], mu[:], rstd[:],
                op0=ALU.subtract, op1=ALU.mult)
            for ds in range(8):
                tp = p1ps.tile([128, 128], BF16, tag="tp", bufs=2)
                nc.tensor.transpose(
                    tp[:], zb[:, ds * 128:(ds + 1) * 128], ident[:])
                nc.vector.tensor_copy(
                    xnt[:, ds, sub * 128:(sub + 1) * 128], tp[:])

        # --- V projection first (per-sub: starts as soon as one sub is up) ---
        for sub in range(SUBS):
            vp = p1ps.tile([128, IL], F32, tag="vp", bufs=3)
            for ds in range(8):
                nc.tensor.matmul(
                    vp[:],
                    xnt[:, ds, sub * 128:(sub + 1) * 128],
                    wv_sb[:, ds, :],
                    start=(ds == 0), stop=(ds == 7))
            tb = c * SUBS + sub
            v_dst = v_sb.rearrange("p b (h c) -> p b h c", c=65)[:, tb, :, 0:64]
            nc.vector.scalar_tensor_tensor(
                v_dst,
                vp.rearrange("p (h c) -> p h c", c=64),
                1.0,
                vb_bc.rearrange("p (h c) -> p h c", c=64),
                op0=ALU.mult, op1=ALU.add)
        # --- Q/K projections (transposed out) ---
        for w_sb, bias_sb, dst in ((wq_sb, qb_sb, qt_sb), (wk_sb, kb_sb, kt_sb)):
            for mb in range(4):
                qp = p1ps.tile([128, CH], F32, tag="qp", bufs=3)
                for ds in range(8):
                    nc.tensor.matmul(
                        qp[:],
                        w_sb[:, ds, mb * 128:(mb + 1) * 128],
                        xnt[:, ds, :],
                        start=(ds == 0), stop=(ds == 7))
                nc.scalar.activation(
                    dst[:, mb, c * CH:(c + 1) * CH], qp[:], AF.Identity,
                    bias=bias_sb[:, mb:mb + 1])

    p1ps_cm.__exit__(None, None, None)

    # ================= phase 2: attention =================
    p2_cm = tc.tile_pool(name="p2", bufs=1)
    p2 = p2_cm.__enter__()
    p2ps_cm = tc.tile_pool(name="p2ps", bufs=1, space="PSUM")
    p2ps = p2ps_cm.__enter__()

    wo_sb = p2.tile([128, 4, D], BF16)
    wo_r = wo_d.rearrange("(a p) m -> p a m", p=128)
    for it in range(4):
        wo_st = p2.tile([128, 1, D], F32, tag="wo_st", bufs=2)
        nc.sync.dma_start(wo_st[:], wo_r[:, it:it + 1, :])
        nc.scalar.copy(wo_sb[:, it:it + 1, :], wo_st[:])

    vv = v_sb.rearrange("p b (h c) -> p b h c", c=65)

    def emit_outproj(tb, chh, pool, bufs):
        op = pool.tile([128, 512], F32, tag="op", bufs=bufs,
                       name=f"op_{tb}_{chh}")
        for it in range(4):
            nc.tensor.matmul(
                op[:],
                ot_sb[:, it, tb * 128:(tb + 1) * 128],
                wo_sb[:, it, chh * 512:(chh + 1) * 512],
                start=(it == 0), stop=(it == 3))
        osb = p2.tile([128, 512], F32, tag="osb", bufs=4)
        nc.vector.tensor_copy(osb[:], op[:])
        nc.sync.dma_start(
            out_d[tb * 128:(tb + 1) * 128,
                  chh * 512:(chh + 1) * 512], osb[:])

    for qb in range(NQB):
        for hp in range(4):
            ota = p2ps.tile([65, QB], F32, tag="ot", bufs=4)
            otb = p2ps.tile([65, QB], F32, tag="ot", bufs=4)

            def emit_st(kb):
                st = p2ps.tile([128, 2 * QB], F32, tag="st", bufs=2,
                               name=f"st_{qb}_{hp}_{kb}")
                nc.tensor.matmul(
                    st[:, 0:QB],
                    kt_sb[0:64, hp, kb * 128:(kb + 1) * 128],
                    qt_sb[0:64, hp, qb * QB:(qb + 1) * QB],
                    start=True, stop=True)
                nc.tensor.matmul(
                    st[:, QB:2 * QB],
                    kt_sb[64:128, hp, kb * 128:(kb + 1) * 128],
                    qt_sb[64:128, hp, qb * QB:(qb + 1) * QB],
                    start=True, stop=True)
                return st

            st_cur = emit_st(0)
            for kb in range(NKB):
                st_next = emit_st(kb + 1) if kb + 1 < NKB else None
                pt = p2.tile([128, 2 * QB], BF16, tag="pt", bufs=4)
                nc.scalar.activation(pt[:], st_cur[:], AF.Exp, scale=SCALE)
                nc.tensor.matmul(
                    ota[:], vv[:, kb, 2 * hp, :], pt[:, 0:QB],
                    start=(kb == 0), stop=(kb == NKB - 1))
                nc.tensor.matmul(
                    otb[:], vv[:, kb, 2 * hp + 1, :], pt[:, QB:2 * QB],
                    start=(kb == 0), stop=(kb == NKB - 1))
                st_cur = st_next
            last_pair = (qb == NQB - 1 and hp == 3)
            for ot, p0 in ((ota, 0), (otb, 64)):
                if last_pair:
                    # quick PSUM->SBUF copy so the attention PSUM pool can
                    # hand its banks to phase 3 ~5us sooner
                    ost = p2.tile([65, QB], F32, tag="ost", bufs=2)
                    nc.vector.tensor_copy(ost[:], ot[:])
                    ot = ost
                recip = p2.tile([1, QB], F32, tag="recip", bufs=2)
                nc.vector.reciprocal(recip[:], ot[64:65, :])
                bc = p2.tile([64, QB], F32, tag="bc", bufs=2)
                nc.gpsimd.partition_broadcast(bc[:], recip[:], channels=64)
                nc.vector.scalar_tensor_tensor(
                    ot_sb[p0:p0 + 64, hp, qb * QB:(qb + 1) * QB],
                    ot[0:64, :], 1.0, bc[:],
                    op0=ALU.mult, op1=ALU.mult)


    p2ps_cm.__exit__(None, None, None)

    # ================= phase 3: out projection (DMA straight from PSUM) ====
    p3ps_cm = tc.tile_pool(name="p3ps", bufs=1, space="PSUM")
    p3ps = p3ps_cm.__enter__()
    for tb in range(NTB):
        for chh in range(2):
            emit_outproj(tb, chh, p3ps, 3)
    p3ps_cm.__exit__(None, None, None)
    p2_cm.__exit__(None, None, None)
    p1_cm.__exit__(None, None, None)


_CACHE = {}


def _get_graph(T=2048):
    if T not in _CACHE:
        _CACHE[T] = build_graph(T=T)
    return _CACHE[T]


def make_in_maps(x, ln_gamma, ln_beta, w_qkv, w_out):
    """Shard full inputs into the 8 per-core input maps."""
    x = np.asarray(x, dtype=np.float32)
    ln_gamma = np.asarray(ln_gamma, dtype=np.float32)
    ln_beta = np.asarray(ln_beta, dtype=np.float32)
    w_qkv = np.asarray(w_qkv, dtype=np.float32)
    w_out = np.asarray(w_out, dtype=np.float32)

    wf = ln_gamma[:, None] * w_qkv                 # gamma folded
    qkv_bias = ln_beta @ w_qkv                     # beta folded
    in_maps = []
    for c in range(8):
        b, hg = c // 2, c % 2
        s = hg * IL
        in_maps.append({
            "x": np.ascontiguousarray(x[b]),
            "wq": np.ascontiguousarray(wf[:, s:s + IL]),
            "wk": np.ascontiguousarray(wf[:, 1024 + s:1024 + s + IL]),
            "wv": np.ascontiguousarray(wf[:, 2048 + s:2048 + s + IL]),
            "wo": np.ascontiguousarray(w_out[s:s + IL, :]),
            "qb": np.ascontiguousarray(qkv_bias[s:s + IL].reshape(4, 128)),
            "kb": np.ascontiguousarray(qkv_bias[1024 + s:1024 + s + IL].reshape(4, 128)),
            "vb": np.ascontiguousarray(qkv_bias[2048 + s:2048 + s + IL].reshape(1, IL)),
        })
    return in_maps


def run(x, ln_gamma, ln_beta, w_qkv, w_out, b_out, trace=False, T=2048):
    nc = _get_graph(T)
    in_maps = make_in_maps(x, ln_gamma, ln_beta, w_qkv, w_out)
    res = bass_utils.run_bass_kernel_spmd(
        nc, in_maps, core_ids=list(range(8)), trace=trace)
    parts = [res.results[c]["out"] for c in range(8)]
    b_out = np.asarray(b_out, dtype=np.float32)
    out = np.stack([parts[2 * b] + parts[2 * b + 1] for b in range(4)])
    out = out + b_out[None, None, :]
    return out.astype(np.float32), res


def kernel(x, ln_gamma, ln_beta, w_qkv, w_out, b_out):
    out, _ = run(x, ln_gamma, ln_beta, w_qkv, w_out, b_out)
    return out
